# revision 3
# baseline (speedup 1.0000x reference)
"""AUGRU (DIEN DynamicGRU) Trainium2 kernel.

Strategy (data-parallel over batch, 8 cores x 32 rows):
  Phase A (precompute): Xg = X @ Wg_x + bg for g in {r,u,h} as big GEMMs
    (f32r, PE-efficient, M=128 tiles), staged to internal DRAM.
  Phase B (recurrence, T sequential steps):
    state h kept natural [32,512] (f32) + transposed hT [128,4,32] (f32r).
    r_pre/u_pre = 4 K-chunk MMs (lhsT=hT chunk, rhs=W_h chunk, N=512)
                  + identity-inject MM adding Xg_t from SBUF.
    sigma/tanh on ScalarE from PSUM; elementwise update on DVE;
    hT_new via 4 PE transposes + ACT copies (f32 -> f32r rounding).

Host side shards/transposes inputs, gathers/transposes outputs.
"""
import sys

sys.path.insert(0, '/opt/trn_rl_repo')

import numpy as np

import concourse.bass as bass
import concourse.tile as tile
from concourse import mybir
from concourse.vector_clock import ScopedClock

F32 = mybir.dt.float32
F32R = mybir.dt.float32r

B, T, D, H = 256, 512, 512, 512
NCORES = 8
BL = B // NCORES  # 32 batch rows per core
KC = 4            # K chunks of 128 over H (and D)

# ---------------------------------------------------------------------------
# toolchain workaround: this walrus build encodes at most ONE sem-wait per
# instruction; spill extra waits onto same-engine nops.
MAXW = 1


def _split_waits_onto_nops(nc, ins):
    si = ins.sync_info
    if si is None or not si.on_wait or len(si.on_wait) <= MAXW:
        return []
    waits = list(si.on_wait)
    keep = waits[:MAXW]
    rest = waits[MAXW:]
    nops = []
    for i in range(0, len(rest), MAXW):
        chunk = rest[i:i + MAXW]
        nop = mybir.InstNoOp(
            name=nc.get_next_instruction_name(),
            ins=[],
            outs=[],
            engine=ins.engine,
            sync_info=mybir.SyncInfo(on_wait=list(chunk), on_update=[]),
        )
        nops.append(nop)
    si.on_wait = keep
    return nops


def _patched_drain_and_barrier(self, tick_clock, wait_clock):
    nc = self.nc
    drain_inst = nc.sync.drain()
    wait_clock.add_sem_waits(
        drain_inst.ins, ScopedClock({None: tick_clock.global_clock})
    )
    ins = drain_inst.ins
    nops = _split_waits_onto_nops(nc, ins)
    if nops:
        bb = nc.cur_bb.bb
        idx = None
        for i, existing in enumerate(bb.instructions):
            if existing is ins:
                idx = i
                break
        assert idx is not None
        for j, nop in enumerate(nops):
            nc.register_instruction(nop, overwrite=True)
            bb.instructions.insert(idx + j, nop)
    nc.all_engine_barrier()
    assert self.sems is not None
    popped = nc._tile_sem_poison_stack.pop()
    assert popped is self._sem_poison
    nc.clear_and_free_semaphores(list(self.sems.allocated().values()))
    nc.all_engine_barrier()


def _split_excess_waits(nc):
    n_fixed = 0
    for f in nc.m.functions:
        for bb in f.blocks:
            i = 0
            insts = bb.instructions
            while i < len(insts):
                nops = _split_waits_onto_nops(nc, insts[i])
                if nops:
                    for j, nop in enumerate(nops):
                        nc.register_instruction(nop, overwrite=True)
                        insts.insert(i + j, nop)
                    i += len(nops)
                    n_fixed += 1
                i += 1
    return n_fixed


tile.TileContext._drain_and_barrier = _patched_drain_and_barrier

# ---------------------------------------------------------------------------


def build(t_steps=T):
    BT = t_steps * BL
    MT = BT // 128  # phase-A output row tiles

    nc = bass.Bass()
    xt = nc.declare_dram_parameter("xt", [D, BT], F32R, isOutput=False)
    av = nc.declare_dram_parameter("av", [t_steps * BL, 1], F32, isOutput=False)
    wr = nc.declare_dram_parameter("wr", [D + H, H], F32R, isOutput=False)
    wu = nc.declare_dram_parameter("wu", [D + H, H], F32R, isOutput=False)
    wh = nc.declare_dram_parameter("wh", [D + H, H], F32R, isOutput=False)
    br = nc.declare_dram_parameter("br", [1, H], F32R, isOutput=False)
    bu = nc.declare_dram_parameter("bu", [1, H], F32R, isOutput=False)
    bh = nc.declare_dram_parameter("bh", [1, H], F32R, isOutput=False)
    i32r = nc.declare_dram_parameter("i32r", [BL, BL], F32R, isOutput=False)
    i32f = nc.declare_dram_parameter("i32f", [BL, BL], F32, isOutput=False)
    ones = nc.declare_dram_parameter("ones", [1, 128], F32R, isOutput=False)
    h0t = nc.declare_dram_parameter("h0t", [128, KC, BL], F32R, isOutput=False)
    out = nc.declare_dram_parameter("out", [t_steps, BL, H], F32, isOutput=True)

    xr_s = nc.dram_tensor("xr_s", [BT, H], F32R)
    xu_s = nc.dram_tensor("xu_s", [BT, H], F32R)
    xh_s = nc.dram_tensor("xh_s", [BT, H], F32R)

    with tile.TileContext(nc) as tc:
        with tc.tile_pool(name="const", bufs=1) as cp:
            # recurrence weights (rows 0:512 of W) and x-part (rows 512:1024)
            w_h = {}
            w_x = {}
            for name, wt in (("r", wr), ("u", wu), ("h", wh)):
                th = cp.tile([128, KC, H], F32R, tag=f"w{name}h")
                nc.sync.dma_start(
                    out=th[:],
                    in_=wt[0:H, :].rearrange("(k p) n -> p k n", p=128),
                )
                w_h[name] = th
                tx = cp.tile([128, KC, H], F32R, tag=f"w{name}x")
                nc.sync.dma_start(
                    out=tx[:],
                    in_=wt[H:H + D, :].rearrange("(k p) n -> p k n", p=128),
                )
                w_x[name] = tx
            bias = {}
            for name, bt_ in (("r", br), ("u", bu), ("h", bh)):
                tb = cp.tile([1, H], F32R, tag=f"b{name}")
                nc.sync.dma_start(out=tb[:], in_=bt_[:])
                bias[name] = tb
            i32r_sb = cp.tile([BL, BL], F32R, tag="i32r")
            nc.sync.dma_start(out=i32r_sb[:], in_=i32r[:])
            i32f_sb = cp.tile([BL, BL], F32, tag="i32f")
            nc.sync.dma_start(out=i32f_sb[:], in_=i32f[:])
            ones_sb = cp.tile([1, 128], F32R, tag="ones")
            nc.sync.dma_start(out=ones_sb[:], in_=ones[:])
            h0t_sb = cp.tile([128, KC, BL], F32R, tag="h0t")
            nc.sync.dma_start(out=h0t_sb[:], in_=h0t[:])

            # ---------------- Phase A: Xg = X @ Wg_x + bg ----------------
            with tc.tile_pool(name="pa_in", bufs=3) as pin, \
                 tc.tile_pool(name="pa_ps", bufs=3, space="PSUM") as pps, \
                 tc.tile_pool(name="pa_out", bufs=3) as pout:
                for m in range(MT):
                    xt_t = pin.tile([128, KC, 128], F32R, tag="xt")
                    nc.sync.dma_start(
                        out=xt_t[:],
                        in_=xt[:, m * 128:(m + 1) * 128].rearrange(
                            "(k p) n -> p k n", p=128
                        ),
                    )
                    for name, stage in (("r", xr_s), ("u", xu_s), ("h", xh_s)):
                        ps = pps.tile([128, H], F32, tag="ps")
                        for k in range(KC):
                            nc.tensor.matmul(
                                ps[:], xt_t[:, k, :], w_x[name][:, k, :],
                                start=(k == 0), stop=False,
                            )
                        nc.tensor.matmul(
                            ps[:], ones_sb[:], bias[name][:],
                            start=False, stop=True,
                        )
                        ob = pout.tile([128, H], F32R, tag="ob")
                        nc.scalar.copy(out=ob[:], in_=ps[:])
                        nc.sync.dma_start(
                            out=stage[m * 128:(m + 1) * 128, :], in_=ob[:]
                        )

            # ---------------- Phase B: recurrence over t ----------------
            with tc.tile_pool(name="pb_xg", bufs=3) as pxg, \
                 tc.tile_pool(name="pb_a", bufs=3) as pa, \
                 tc.tile_pool(name="pb_psg", bufs=3, space="PSUM") as psg, \
                 tc.tile_pool(name="pb_pst", bufs=2, space="PSUM") as pst, \
                 tc.tile_pool(name="pb_sb", bufs=2) as psb, \
                 tc.tile_pool(name="pb_ht", bufs=2) as pht:
                h_nat = psb.tile([BL, H], F32, tag="h_nat")
                nc.vector.memset(h_nat[:], 0.0)
                h_t = h0t_sb

                for t in range(t_steps):
                    xg_t = {}
                    for name, stage in (("r", xr_s), ("u", xu_s), ("h", xh_s)):
                        xg = pxg.tile([BL, H], F32R, tag=f"x{name}")
                        nc.sync.dma_start(
                            out=xg[:], in_=stage[t * BL:(t + 1) * BL, :]
                        )
                        xg_t[name] = xg
                    a_t = pa.tile([BL, 1], F32, tag="a")
                    nc.sync.dma_start(out=a_t[:], in_=av[t * BL:(t + 1) * BL, :])

                    # r and u gates
                    ps_r = psg.tile([BL, H], F32, tag="psg")
                    for k in range(KC):
                        nc.tensor.matmul(
                            ps_r[:], h_t[:, k, :], w_h["r"][:, k, :],
                            start=(k == 0), stop=False,
                        )
                    nc.tensor.matmul(
                        ps_r[:], i32r_sb[:], xg_t["r"][:], start=False, stop=True
                    )
                    ps_u = psg.tile([BL, H], F32, tag="psg")
                    for k in range(KC):
                        nc.tensor.matmul(
                            ps_u[:], h_t[:, k, :], w_h["u"][:, k, :],
                            start=(k == 0), stop=False,
                        )
                    nc.tensor.matmul(
                        ps_u[:], i32r_sb[:], xg_t["u"][:], start=False, stop=True
                    )
                    r_sb = psb.tile([BL, H], F32, tag="r")
                    nc.scalar.activation(
                        r_sb[:], ps_r[:], mybir.ActivationFunctionType.Sigmoid
                    )
                    u_sb = psb.tile([BL, H], F32, tag="u")
                    nc.scalar.activation(
                        u_sb[:], ps_u[:], mybir.ActivationFunctionType.Sigmoid
                    )

                    # hr = h * r, transposed chunks for the h_hat matmul
                    hr_sb = psb.tile([BL, H], F32, tag="hr")
                    nc.vector.tensor_mul(hr_sb[:], h_nat[:], r_sb[:])
                    hrt = pht.tile([128, KC, BL], F32R, tag="hrt")
                    for k in range(KC):
                        tp = pst.tile([128, BL], F32, tag="tp")
                        nc.tensor.transpose(
                            tp[:], hr_sb[:, k * 128:(k + 1) * 128], i32f_sb[:]
                        )
                        nc.scalar.copy(out=hrt[:, k, :], in_=tp[:])

                    ps_h = psg.tile([BL, H], F32, tag="psg")
                    for k in range(KC):
                        nc.tensor.matmul(
                            ps_h[:], hrt[:, k, :], w_h["h"][:, k, :],
                            start=(k == 0), stop=False,
                        )
                    nc.tensor.matmul(
                        ps_h[:], i32r_sb[:], xg_t["h"][:], start=False, stop=True
                    )
                    hh_sb = psb.tile([BL, H], F32, tag="hh")
                    nc.scalar.activation(
                        hh_sb[:], ps_h[:], mybir.ActivationFunctionType.Tanh
                    )

                    # h_new = h + (a*u) * (hh - h)
                    ua_sb = psb.tile([BL, H], F32, tag="ua")
                    nc.vector.tensor_scalar_mul(ua_sb[:], u_sb[:], a_t[:])
                    d_sb = psb.tile([BL, H], F32, tag="d")
                    nc.vector.tensor_sub(d_sb[:], hh_sb[:], h_nat[:])
                    m_sb = psb.tile([BL, H], F32, tag="m")
                    nc.vector.tensor_mul(m_sb[:], ua_sb[:], d_sb[:])
                    hn_sb = psb.tile([BL, H], F32, tag="h_nat")
                    nc.vector.tensor_add(hn_sb[:], h_nat[:], m_sb[:])

                    nc.sync.dma_start(out=out[t, :, :], in_=hn_sb[:])

                    # transposed state for next step
                    if t != t_steps - 1:
                        ht_new = pht.tile([128, KC, BL], F32R, tag="ht")
                        for k in range(KC):
                            tp = pst.tile([128, BL], F32, tag="tp")
                            nc.tensor.transpose(
                                tp[:], hn_sb[:, k * 128:(k + 1) * 128], i32f_sb[:]
                            )
                            nc.scalar.copy(out=ht_new[:, k, :], in_=tp[:])
                        h_t = ht_new
                    h_nat = hn_sb

    _split_excess_waits(nc)
    return nc


_BUILD_CACHE = {}


def _get_built(t_steps):
    if t_steps not in _BUILD_CACHE:
        _BUILD_CACHE[t_steps] = build(t_steps)
    return _BUILD_CACHE[t_steps]


def make_in_maps(X, attention_scores, Wr, br, Wu, bu, Wh, bh, t_steps=T):
    shared = {
        "wr": np.ascontiguousarray(Wr, dtype=np.float32),
        "wu": np.ascontiguousarray(Wu, dtype=np.float32),
        "wh": np.ascontiguousarray(Wh, dtype=np.float32),
        "br": np.ascontiguousarray(br, dtype=np.float32).reshape(1, H),
        "bu": np.ascontiguousarray(bu, dtype=np.float32).reshape(1, H),
        "bh": np.ascontiguousarray(bh, dtype=np.float32).reshape(1, H),
        "i32r": np.eye(BL, dtype=np.float32),
        "i32f": np.eye(BL, dtype=np.float32),
        "ones": np.ones((1, 128), dtype=np.float32),
        "h0t": np.zeros((128, KC, BL), dtype=np.float32),
    }
    in_maps = []
    for c in range(NCORES):
        bs = slice(c * BL, (c + 1) * BL)
        xc = np.asarray(X[bs, :t_steps, :], dtype=np.float32)   # [BL, t, D]
        xt = np.ascontiguousarray(
            xc.transpose(2, 1, 0).reshape(D, t_steps * BL)
        )                                                       # [D, t*BL]
        ac = np.ascontiguousarray(
            np.asarray(attention_scores[bs, :t_steps], dtype=np.float32).T
        ).reshape(t_steps * BL, 1)                              # [t*BL, 1]
        in_maps.append({"xt": xt, "av": ac, **shared})
    return in_maps


def kernel(X, attention_scores, Wr, br, Wu, bu, Wh, bh):
    from concourse.bass_utils import run_bass_kernel_spmd

    nc = _get_built(T)
    in_maps = make_in_maps(X, attention_scores, Wr, br, Wu, bu, Wh, bh, T)
    res = run_bass_kernel_spmd(nc, in_maps, core_ids=list(range(NCORES)))
    out = np.empty((B, T, H), dtype=np.float32)
    for c in range(NCORES):
        bs = slice(c * BL, (c + 1) * BL)
        out[bs] = res.results[c]["out"].transpose(1, 0, 2)
    return out
